# revision 13
# baseline (speedup 1.0000x reference)
"""Single-directional Chamfer distance (pytorch3d semantics) on 8 trn2 cores.

loss = mean_b mean_i min_j ||x_bi - y_bj||^2   with x = v_pred, y = v.

Sharding: batch B=8 across the 8 cores, one point-cloud pair per core.

Candidate-pruned algorithm (device does O(N * K) instead of O(N^2)):
  Host partitions targets into 256 equal-mass kd cells (64 pts each) and
  queries into 128 groups (128 each).  Each group gets the NC=64 cells
  nearest its bounding box -> K=4096 candidate targets.  The device
  computes per-query min d2 over the candidates only.  A per-query
  CERTIFICATE makes the result exact: min d2 is provably correct if it is
  below the min box-distance^2 to every EXCLUDED cell; the handful of
  queries failing the certificate (~1-2 per core) are recomputed exactly
  on the host.

  d2 is computed DIRECTLY in PSUM by a K=16 bf16 matmul over augmented
  coordinates with every fp32 operand split into (hi, lo) bf16 parts
  (residual ~2^-18 per operand), 4x faster than fp32 matmuls:
     d2[i,j] = xsq_i + ysq_j - 2 x_i.y_j
     rows: (xsqh,xsql)x(1,1), (1,1)x(ysqh,ysql), (-2x hi/lo)x(y hi/lo).

Raw bass implementation (explicit semaphores): PE/DVE ping-pong over two
4-bank PSUM buffers; candidate tiles stream from DRAM double-buffered.
"""

import os

import numpy as np
from ml_dtypes import bfloat16

import concourse.bass as bass
import concourse.mybir as mybir
from concourse.bass_utils import run_bass_kernel_spmd

F32 = mybir.dt.float32
BF16 = mybir.dt.bfloat16
N = 16384
NCORES = 8
KAUG = 16

NQB = N // 128       # 128 query groups
NCELL = 256          # target kd cells
CSZ = N // NCELL     # 64 targets per cell
NC = 64              # candidate cells per group
KC = NC * CSZ        # 4096 candidate targets per group
TBS = 2048           # targets per PSUM tile (4 banks)
NTB = KC // TBS      # 2 tiles per group
NT = NQB * NTB       # 256 tiles
YCW = NQB * KC       # candidate stream width

_BUILD_CACHE = {}


def _build():
    nc = bass.Bass()
    xa = nc.dram_tensor("xa", [KAUG, N], BF16, kind="ExternalInput")
    yc = nc.dram_tensor("yc", [KAUG, YCW], BF16, kind="ExternalInput")
    out = nc.dram_tensor("out", [128, NQB], F32, kind="ExternalOutput")

    AX = mybir.AxisListType
    OP = mybir.AluOpType

    with (
        nc.sbuf_tensor([KAUG, N], BF16) as lhsT,
        nc.sbuf_tensor([KAUG, TBS], BF16) as rbA,
        nc.sbuf_tensor([KAUG, TBS], BF16) as rbB,
        nc.sbuf_tensor([128, NTB], F32) as mcols,
        nc.sbuf_tensor([128, NQB], F32) as m_grid,
        nc.psum_tensor([128, TBS], F32) as psA,
        nc.psum_tensor([128, TBS], F32) as psB,
        nc.semaphore() as dma_sem,
        nc.semaphore() as mm_sem,
        nc.semaphore() as red_sem,
        nc.semaphore() as qb_sem,
        nc.Block() as block,
    ):

        @block.sync
        def _(sync):
            sync.dma_start(lhsT[:, :], xa[:, :]).then_inc(dma_sem, 16)
            for tidx in range(NT):
                rb = rbA if tidx % 2 == 0 else rbB
                if tidx >= 2:
                    # PE finished the matmuls of tile tidx-2 -> rb free
                    sync.wait_ge(mm_sem, tidx - 1)
                    # chain: all prior DMAs done, so dma_sem totals are
                    # unambiguous for the PE's waits (DMA rings complete
                    # out of order; only one rb transfer in flight)
                    sync.wait_ge(dma_sem, 16 * (tidx + 1))
                sync.dma_start(
                    rb[:, :], yc[:, tidx * TBS : (tidx + 1) * TBS]
                ).then_inc(dma_sem, 16)
            sync.wait_ge(qb_sem, NQB)
            sync.dma_start(out[:, :], m_grid[:, :]).then_inc(dma_sem, 16)

        @block.tensor
        def _(tensor):
            for tidx in range(NT):
                qb = tidx // NTB
                rb = rbA if tidx % 2 == 0 else rbB
                ps = psA if tidx % 2 == 0 else psB
                # rhs tile DMA done (16 for xa + 16 per tile).  Tiles 0/1
                # wait for all three startup DMAs: with both rb DMAs in
                # flight at once their completions can arrive out of order.
                # For t>=2 the mm_sem gate on the DMA queue bounds in-flight
                # transfers to tiles <= t, so the running total is exact.
                tensor.wait_ge(dma_sem, max(48, 16 * (tidx + 2)))
                if tidx >= 2:
                    # reduce of tile tidx-2 released this psum buffer
                    tensor.wait_ge(red_sem, tidx - 1)
                lw = lhsT[:, qb * 128 : (qb + 1) * 128]
                for k in range(TBS // 512):
                    mm = nc.tensor.matmul(
                        ps[:, k * 512 : (k + 1) * 512],
                        lw,
                        rb[:, k * 512 : (k + 1) * 512],
                        start=True,
                        stop=True,
                    )
                mm.then_inc(mm_sem, 1)

        @block.vector
        def _(vector):
            for tidx in range(NT):
                qb, tb = divmod(tidx, NTB)
                ps = psA if tidx % 2 == 0 else psB
                vector.wait_ge(mm_sem, tidx + 1)
                if tb == 0 and qb > 0:
                    # WAR: previous qb's second-stage read of mcols
                    vector.wait_ge(qb_sem, qb)
                nc.vector.tensor_reduce(
                    mcols[:, tb : tb + 1], ps[:, :], axis=AX.X, op=OP.min
                ).then_inc(red_sem, 1)
                if tb == NTB - 1:
                    # red_sem counts completed stage-1 reduces
                    vector.wait_ge(red_sem, tidx + 1)
                    nc.vector.tensor_reduce(
                        m_grid[:, qb : qb + 1], mcols[:, :], axis=AX.X, op=OP.min
                    ).then_inc(qb_sem, 1)

    return nc


def _split(a):
    hi = a.astype(bfloat16)
    lo = (a - hi.astype(np.float64)).astype(bfloat16)
    return hi, lo


def _aug16(p64):
    """[M,3] float64 points -> [16, M] bf16 augmented target rows."""
    M = p64.shape[0]
    sqh, sql = _split((p64 * p64).sum(axis=1))
    ph, pl = _split(p64)
    a = np.empty((KAUG, M), bfloat16)
    a[0] = 1.0
    a[1] = 1.0
    a[2] = sqh
    a[3] = sql
    a[4:7] = ph.T
    a[7:10] = pl.T
    a[10:13] = ph.T
    a[13:16] = pl.T
    return a


def _kdperm(p, s0, s1):
    """Equal-mass kd permutation: s0 x-slabs, s1 y-rows each, z-sorted cells."""
    perm = np.argsort(p[:, 0], kind="stable")
    n0 = len(p) // s0
    for s in range(s0):
        sl = perm[s * n0 : (s + 1) * n0]
        sl = sl[np.argsort(p[sl, 1], kind="stable")]
        n1 = n0 // s1
        for r in range(s1):
            rw = sl[r * n1 : (r + 1) * n1]
            rw = rw[np.argsort(p[rw, 2], kind="stable")]
            sl[r * n1 : (r + 1) * n1] = rw
        perm[s * n0 : (s + 1) * n0] = sl
    return perm


def _marshal_core(x32, y32):
    """Build device operands + certificate data for one core."""
    x, y = x32.astype(np.float64), y32.astype(np.float64)
    qp = _kdperm(x, 8, 4)
    tp = _kdperm(y, 8, 8)
    xg = x[qp].reshape(NQB, 128, 3)
    yg = y[tp].reshape(NCELL, CSZ, 3)
    qlo, qhi = xg.min(1), xg.max(1)
    tlo, thi = yg.min(1), yg.max(1)
    qc, tc = xg.mean(1), yg.mean(1)
    gap = np.maximum(
        0, np.maximum(qlo[:, None] - thi[None], tlo[None] - qhi[:, None])
    )
    dist = (gap**2).sum(-1) + 1e-6 * ((qc[:, None] - tc[None]) ** 2).sum(-1)
    cand = np.argsort(dist, axis=1)[:, :NC]  # [NQB, NC]

    # per-query lower-bound certificate vs excluded cells (float32 is
    # plenty: the certificate compare carries 2% slack)
    q32 = x[qp].astype(np.float32)
    tlo32, thi32 = tlo.astype(np.float32), thi.astype(np.float32)
    qcd = np.zeros((N, NCELL), np.float32)
    for dim in range(3):
        g0 = np.maximum(
            0,
            np.maximum(
                q32[:, dim : dim + 1] - thi32[None, :, dim],
                tlo32[None, :, dim] - q32[:, dim : dim + 1],
            ),
        )
        qcd += g0 * g0
    qcd = qcd.reshape(NQB, 128, NCELL)
    for g in range(NQB):
        qcd[g][:, cand[g]] = np.inf
    lb = qcd.min(2)

    # augmented bf16 operands, translated per group to the group centroid
    # (d2 is translation-invariant; small coordinates shrink the bf16
    # hi/lo split residual by ~100x)
    xrel = (xg - qc[:, None, :]).reshape(N, 3)
    xa = np.empty((KAUG, N), bfloat16)
    sqh, sql = _split((xrel * xrel).sum(axis=1))
    m2h, m2l = _split(-2.0 * xrel)
    xa[0] = sqh
    xa[1] = sql
    xa[2] = 1.0
    xa[3] = 1.0
    xa[4:7] = m2h.T
    xa[7:10] = m2h.T
    xa[10:13] = m2l.T
    xa[13:16] = m2l.T

    col = (cand[:, :, None] * CSZ + np.arange(CSZ)[None, None, :]).reshape(
        NQB, KC
    )
    ygath = y[tp][col.ravel()].reshape(NQB, KC, 3) - qc[:, None, :]
    yrel = ygath.reshape(NQB * KC, 3)
    yc_arr = _aug16(yrel)            # [16, YCW]

    return {"xa": xa, "yc": yc_arr}, qp, lb.ravel(), x, y


def kernel(v: np.ndarray, v_pred: np.ndarray) -> np.ndarray:
    v = np.ascontiguousarray(np.asarray(v, dtype=np.float32))
    v_pred = np.ascontiguousarray(np.asarray(v_pred, dtype=np.float32))
    assert v.shape == (NCORES, N, 3) and v_pred.shape == (NCORES, N, 3)

    if "k" not in _BUILD_CACHE:
        _BUILD_CACHE["k"] = _build()
    nc = _BUILD_CACHE["k"]

    in_maps, meta = [], []
    for b in range(NCORES):
        im, qp, lb, x, y = _marshal_core(v_pred[b], v[b])
        in_maps.append(im)
        meta.append((qp, lb, x, y))

    res = run_bass_kernel_spmd(
        nc,
        in_maps,
        core_ids=list(range(NCORES)),
        trace=bool(int(os.environ.get("BASS_TRACE_KERNEL", "0"))),
    )
    if res.exec_time_ns is not None:
        print(f"HW exec time: {res.exec_time_ns} ns")

    per_core = []
    for b, r in enumerate(res.results):
        qp, lb, x, y = meta[b]
        mins = np.asarray(r["out"], dtype=np.float64).T.ravel()  # [N] in qp order
        # certificate: candidate min must beat the best excluded cell bound;
        # recompute the few failures exactly on host
        need = mins >= lb * 0.98 - 1e-4
        if need.any():
            xo = x[qp][need]
            ys_ = (y * y).sum(1)
            d2o = (xo * xo).sum(1)[:, None] - 2.0 * (xo @ y.T) + ys_[None, :]
            mins[need] = d2o.min(1)
        per_core.append(mins.mean())
    loss = np.float32(np.mean(per_core))
    return np.array(loss, dtype=np.float32)
